# revision 47
# baseline (speedup 1.0000x reference)
"""Trainium2 Bass kernel for DGC-style GNN message passing (8 NeuronCores).

Model (matches the jax reference):
    h = x @ emb_W + emb_b
    row/col/norm = gcn_norm_improved(edge_index)   (self-loop weight 2.0)
    4x: h = h - eps * segment_sum(norm * h[row], col)
    h = tanh(h)
    per-graph pooling [sum | max | mean]  ->  2-layer leaky-relu MLP -> [G, 32]

Distribution: nodes are sharded across the 8 cores by *graph* (8 graphs per
core), with every graph padded to a fixed W=1024 slot window so the program
structure is identical on every core (SPMD).  Each iteration the cores
all-gather a degree-prescaled bf16 copy of h (hs = deg^-1/2 * h), gather the
source rows of their local edges with SWDGE dma_gather, and scatter-add into
their 128-target-node blocks with one-hot matmuls on the PE.  The norm factors
are folded into per-partition scalars:
    h_new[t] = (1 - 2*eps*dis[t]^2) * h[t] - eps*dis[t] * sum_e dis[src] h[src]

Balance: graphs are assigned to (core, window) positions by edge count
(similar-size graphs share a window position across cores) and nodes are
assigned to their graph's 8 blocks balancing per-block lo/hi in-degree, so
the per-(block, src-half) gather runs are near-uniform across cores and the
core-uniform SPMD padding (pad runs to the max over cores) is ~1-2%.

Edge layout: per core, edges sort by (target block, src-table half); each
(block, half) run is padded to the max-over-cores length in GRAN-slot units
and packed contiguously into two gather streams (lo/hi table halves, int16
dma_gather indices).  A 128-edge tile can straddle adjacent blocks; each
(tile, block) pair gets its own masked one-hot column.  Gathers run as
CHUNK-index dma_gather instructions; one-hot builds split DVE/GPSIMD;
per-partition scale ops run on ACT/DVE.
"""

import os
from contextlib import ExitStack
from dataclasses import dataclass, field

import numpy as np
import ml_dtypes

import concourse.bass as bass
import concourse.bacc as bacc
import concourse.tile as tile
from concourse import mybir
from concourse import bass_utils

dt = mybir.dt
BF16 = ml_dtypes.bfloat16
AX = mybir.AxisListType
OP = mybir.AluOpType
ACTF = mybir.ActivationFunctionType

# ---------------------------------------------------------------- constants
N_NODES = 50000
N_EDGES = 800000
N_GRAPHS = 64
IN_DIM = 128
HID = 128
OUT_DIM = 32
EPSILON = 0.1
ITERATIONS = 4

NCORES = 8
SLOT_W = 1024          # padded slot window per graph
GPC = N_GRAPHS // NCORES   # graphs per core
NPC = GPC * SLOT_W         # padded nodes per core
NBLK = NPC // 128          # 128-node blocks per core
NT = NCORES * NPC          # total padded nodes
HALF = NT // 2             # gather-table half size (int16 index limit)
CHUNK = int(os.environ.get("KERNEL_CHUNK", "8192"))  # gather idxs per dma_gather
TLSIM = bool(int(os.environ.get("KERNEL_TLSIM", "0")))   # cost-model probe build
OHSHARE = int(os.environ.get("KERNEL_OHSHARE", "3"))  # every Nth onehot -> gpsimd
GRAN = int(os.environ.get("KERNEL_GRAN", "1"))        # stream packing granularity
WCHUNKS = int(os.environ.get("KERNEL_WCHUNKS", "8"))  # hs_shard write chunks

# packed f32 const columns: dis | a | b | padneg | ident | embb | emask | invcnt
_C_DIS = 0
_C_A = NBLK
_C_B = 2 * NBLK
_C_PAD = 3 * NBLK
_C_ID = 4 * NBLK
_C_EMBB = 4 * NBLK + 128
_C_EMASK = 4 * NBLK + 256
_C_INV = 4 * NBLK + 256 + GPC
_C_F32 = _C_INV + 1
# packed bf16 const columns: ghot | iota | embW | identbf
_B_GHOT = 0
_B_IOTA = NBLK * GPC
_B_EMBW = NBLK * GPC + 128
_B_IDBF = NBLK * GPC + 256
_B_BF = NBLK * GPC + 384


# ---------------------------------------------------------------- host prep
@dataclass
class Prep:
    """Per-problem preprocessed metadata + per-core input arrays."""
    n_lo: int = 0                 # padded lo-stream length (indices)
    n_hi: int = 0
    ntiles: int = 0               # total edge tiles (consumed by matmuls)
    # per block: list of (stream(0/1), stream_tile_pos, global_tile_idx)
    block_tiles: list = field(default_factory=list)
    in_maps: list = field(default_factory=list)
    pos_of: np.ndarray | None = None   # graph -> position (core*GPC + w)


def _bf(x):
    return np.ascontiguousarray(x.astype(BF16))


def preprocess(x, edge_index, batch, emb_W, emb_b, W1, b1, W2, b2):
    x = np.asarray(x, np.float32)
    edge_index = np.asarray(edge_index, np.int32)
    batch = np.asarray(batch, np.int32)

    G, W, D = N_GRAPHS, SLOT_W, HID
    N = x.shape[0]

    starts = np.searchsorted(batch, np.arange(G + 1)).astype(np.int64)
    cnt = np.diff(starts)
    assert cnt.max() <= W, f"graph size {cnt.max()} exceeds slot window {W}"

    row = edge_index[0].astype(np.int64)
    col = edge_index[1].astype(np.int64)
    deg = (np.bincount(col, minlength=N).astype(np.float32) + 2.0)
    dis = (1.0 / np.sqrt(np.maximum(deg, 1e-30))).astype(np.float32)  # [N]

    # ---------------- graph -> (core, window) position assignment
    # Sort graphs by incident-edge count; window position w gets ranks
    # [8w, 8w+8) (similar sizes share a position across cores, shrinking the
    # max-over-cores run padding), assigned to cores greedily to balance the
    # per-core edge totals.
    tgt_g = batch[col].astype(np.int64)
    E_g = np.bincount(tgt_g, minlength=G).astype(np.int64)
    order_g = np.argsort(-E_g, kind="stable")
    pos_of = np.empty(G, np.int64)
    core_load = np.zeros(NCORES, np.int64)
    for w in range(GPC):
        grp = order_g[w * NCORES:(w + 1) * NCORES]   # desc within group
        cs = np.argsort(core_load, kind="stable")    # asc load
        for i, g in enumerate(grp):
            c = cs[i]
            pos_of[g] = c * GPC + w
            core_load[c] += E_g[g]
    graph_at = np.empty(G, np.int64)
    graph_at[pos_of] = np.arange(G)

    # ---------------- node -> slot assignment (balance lo/hi in-degree
    # across each graph's 8 blocks)
    core_of_g = pos_of // GPC
    lo_node = core_of_g[batch.astype(np.int64)] < (NCORES // 2)
    lo_e = lo_node[row]
    indeg_lo = np.bincount(col[lo_e], minlength=N).astype(np.int64)
    indeg_hi = np.bincount(col[~lo_e], minlength=N).astype(np.int64)

    BPG = W // 128          # blocks per graph window
    slot = np.empty(N, np.int64)
    for g in range(G):
        nodes = np.arange(starts[g], starts[g + 1])
        p = pos_of[g]
        base = (p // GPC) * NPC + (p % GPC) * W
        l = indeg_lo[nodes]
        h = indeg_hi[nodes]
        order_n = np.argsort(-(l + h), kind="stable")
        bl = np.zeros(BPG)
        bh = np.zeros(BPG)
        bc = np.zeros(BPG, np.int64)
        blk_of = np.empty(len(nodes), np.int64)
        for i in order_n:
            costs = (bl + l[i]) ** 2 + (bh + h[i]) ** 2
            costs[bc >= 128] = np.inf
            b = int(np.argmin(costs))
            blk_of[i] = b
            bl[b] += l[i]
            bh[b] += h[i]
            bc[b] += 1
        off = np.empty(len(nodes), np.int64)
        for b in range(BPG):
            m = blk_of == b
            off[m] = np.arange(int(m.sum()))
        slot[nodes] = base + blk_of * 128 + off

    node_of_slot = np.full(NT, -1, np.int64)
    node_of_slot[slot] = np.arange(N, dtype=np.int64)
    real = node_of_slot >= 0                                       # [NT]

    # per-slot vectors, [NT]
    dis_s = np.where(real, dis[np.maximum(node_of_slot, 0)], 0.0).astype(np.float32)
    a_s = np.where(real, 1.0 - 2.0 * EPSILON * dis_s * dis_s, 0.0).astype(np.float32)
    b_s = np.where(real, -EPSILON * dis_s, 0.0).astype(np.float32)
    padneg_s = np.where(real, 0.0, -30.0).astype(np.float32)

    # ---------------- edges -> (core, block) tiles
    src_slot = slot[row]
    dst_slot = slot[col]
    core = dst_slot // NPC
    lloc = dst_slot % NPC
    blk = lloc // 128
    tloc = (lloc % 128).astype(np.float32)

    # gather-table rows are (core, partition, block)-major so the hs_shard
    # write from SBUF [p, (b f)] is a contiguous DMA: row(slot) =
    # core*NPC + (slot%128)*NBLK + (slot%NPC)//128
    sll = src_slot % NPC
    src_row = (src_slot // NPC) * NPC + (sll % 128) * NBLK + sll // 128
    half = (src_row >= HALF).astype(np.int64)
    key = (core * NBLK + blk) * 2 + half
    counts = np.bincount(key, minlength=NCORES * NBLK * 2).reshape(NCORES, NBLK, 2)
    # pad each (block, half) run to the max over cores in GRAN-slot units so
    # the SPMD program is core-uniform
    R = -(-counts.max(axis=0) // GRAN)         # [NBLK, 2] GRAN-units per run
    spt = 128 // GRAN                          # GRAN-units per 128-edge tile
    sb_lo = np.zeros(NBLK + 1, np.int64)       # run bases per stream (GRAN units)
    sb_hi = np.zeros(NBLK + 1, np.int64)
    sb_lo[1:] = np.cumsum(R[:, 0])
    sb_hi[1:] = np.cumsum(R[:, 1])
    nt_lo = int(-(-sb_lo[-1] // spt))          # stream tiles
    nt_hi = int(-(-sb_hi[-1] // spt))

    tpc = CHUNK // 128
    nt_lo_p = max(-(-nt_lo // tpc) * tpc, tpc)
    nt_hi_p = max(-(-nt_hi // tpc) * tpc, tpc)

    # per block: list of (stream, stream_tile_pos, colloc_col); a tile shared
    # by several blocks needs a separate masked one-hot column per block.
    block_tiles = []
    pair_col = {}
    col_idx = 0
    for b in range(NBLK):
        ents = []
        for s, sb in ((0, sb_lo), (1, sb_hi)):
            if sb[b + 1] > sb[b]:
                t0 = int(sb[b]) // spt
                t1 = int(sb[b + 1] - 1) // spt
                for t in range(t0, t1 + 1):
                    pair_col[(s, b, t)] = col_idx
                    ents.append((s, t, col_idx))
                    col_idx += 1
        block_tiles.append(ents)
    ntiles = col_idx

    # order edges by (core, blk, half) once; then per-core slices
    order = np.argsort(key, kind="stable")
    key_sorted = key[order]
    grp_start = np.searchsorted(key_sorted, np.arange(NCORES * NBLK * 2))
    within = np.arange(len(order), dtype=np.int64) - grp_start[key_sorted]

    emb_W = np.asarray(emb_W, np.float32)
    emb_b = np.asarray(emb_b, np.float32)
    W1 = np.asarray(W1, np.float32)
    b1 = np.asarray(b1, np.float32)
    W2 = np.asarray(W2, np.float32)
    b2 = np.asarray(b2, np.float32)
    H2 = W1.shape[1]            # 3*HID//2 = 192

    iota = np.tile(np.arange(128, dtype=np.float32), (128, 1))
    ident = np.eye(128, dtype=np.float32)
    ones_row = np.ones((1, 128), np.float32)

    cnt_pos = cnt[graph_at].astype(np.float32)      # per position
    invcnt = (1.0 / np.maximum(cnt_pos, 1.0)).astype(np.float32)

    in_maps = []
    for k in range(NCORES):
        sl0 = k * NPC
        sel = slice(sl0, sl0 + NPC)
        # [128, NBLK] per-partition-scalar layouts: value at (p, b) = slot b*128+p
        def colmajor(v):
            return np.ascontiguousarray(v[sel].reshape(NBLK, 128).T.astype(np.float32))

        rl = real[sel]

        # packed f32 consts [128, _C_F32]
        scal = np.zeros((128, _C_F32), np.float32)
        scal[:, _C_DIS:_C_DIS + NBLK] = colmajor(dis_s)
        scal[:, _C_A:_C_A + NBLK] = colmajor(a_s)
        scal[:, _C_B:_C_B + NBLK] = colmajor(b_s)
        scal[:, _C_PAD:_C_PAD + NBLK] = colmajor(padneg_s)
        scal[:, _C_ID:_C_ID + 128] = ident
        scal[:, _C_EMBB:_C_EMBB + 128] = np.tile(emb_b, (128, 1))
        # emask: 0 for empty graphs of this core (zero the max), else 1
        scal[:, _C_EMASK:_C_EMASK + GPC] = np.tile(
            (cnt_pos[k * GPC:(k + 1) * GPC] > 0).astype(np.float32), (128, 1))
        scal[0:GPC, _C_INV] = invcnt[k * GPC:(k + 1) * GPC]

        # packed bf16 consts [128, _B_BF]
        bfc = np.zeros((128, _B_BF), np.float32)
        # ghot: one-hot graph assignment, excludes pads
        ghot = np.zeros((NBLK, 128, GPC), np.float32)
        gg_of_blk = np.arange(NBLK) // BPG
        ghot[np.arange(NBLK), :, gg_of_blk] = rl.reshape(NBLK, 128).astype(np.float32)
        bfc[:, _B_GHOT:_B_GHOT + NBLK * GPC] = ghot.transpose(1, 0, 2).reshape(128, NBLK * GPC)
        bfc[:, _B_IOTA:_B_IOTA + 128] = iota
        bfc[:, _B_EMBW:_B_EMBW + 128] = emb_W
        bfc[:, _B_IDBF:_B_IDBF + 128] = ident

        # xT [128, NPC] bf16 (features on partitions)
        xT = np.zeros((D, NPC), np.float32)
        xT[:, rl] = x[node_of_slot[sel][rl]].T
        xT = _bf(xT)

        # edge index streams + col_local
        lo_stream = np.zeros(nt_lo_p * 128, np.int64)
        hi_stream = np.zeros(nt_hi_p * 128, np.int64)
        colloc = np.full((128, ntiles), -1.0, np.float32)

        m = core[order] == k
        o = order[m]
        ks = key_sorted[m]
        w = within[m]
        b_e = (ks // 2) % NBLK
        h_e = ks % 2
        lo_m = h_e == 0
        # stream position = run base (GRAN units) * GRAN + within-run position
        spos = np.where(lo_m, sb_lo[b_e], sb_hi[b_e]) * GRAN + w
        part = spos % 128
        stile = spos // 128
        lo_stream[spos[lo_m]] = src_row[o][lo_m]
        hi_stream[spos[~lo_m]] = src_row[o][~lo_m] - HALF
        cc = np.fromiter(
            (pair_col[(int(h), int(b), int(t))]
             for h, b, t in zip(h_e, b_e, stile)),
            dtype=np.int64, count=len(o))
        colloc[part, cc] = tloc[o]

        def i16_arr(stream):
            # dma_gather layout: idx i -> (i%16, i//16), replicated x8
            a = stream.reshape(-1, 16).T.astype(np.int16)
            return np.ascontiguousarray(np.tile(a, (8, 1)))

        in_maps.append({
            "xT": xT,
            "idxlo16": i16_arr(lo_stream), "idxhi16": i16_arr(hi_stream),
            "colloc": np.ascontiguousarray(colloc),
            "scal": np.ascontiguousarray(scal),
            "bfc": _bf(bfc),
            "ones_bf": _bf(ones_row),
            "W1": _bf(W1), "b1": _bf(b1.reshape(1, H2)),
            "W2": _bf(W2), "b2": _bf(b2.reshape(1, OUT_DIM)),
        })

    prep = Prep(n_lo=nt_lo_p * 128, n_hi=nt_hi_p * 128, ntiles=ntiles,
                block_tiles=block_tiles, in_maps=in_maps, pos_of=pos_of)
    prep.nt_lo = nt_lo
    prep.nt_hi = nt_hi
    return prep


# ---------------------------------------------------------------- program
def build_program(prep: Prep):
    nc = bacc.Bacc("TRN2", target_bir_lowering=False, debug=False,
                   num_devices=(1 if TLSIM else NCORES))
    D = HID
    H2 = 3 * HID // 2
    NLO, NHI, NTILES = prep.n_lo, prep.n_hi, prep.ntiles
    TPC = CHUNK // 128                 # tiles per gather chunk

    def inp(name, shape, d):
        return nc.dram_tensor(name, shape, d, kind="ExternalInput")

    xT_d = inp("xT", [D, NPC], dt.bfloat16)
    idxlo16_d = inp("idxlo16", [128, NLO // 16], dt.int16)
    idxhi16_d = inp("idxhi16", [128, NHI // 16], dt.int16)
    colloc_d = inp("colloc", [128, NTILES], dt.float32)
    scal_d = inp("scal", [128, _C_F32], dt.float32)
    bfc_d = inp("bfc", [128, _B_BF], dt.bfloat16)
    ones_d = inp("ones_bf", [1, 128], dt.bfloat16)
    W1_d = inp("W1", [3 * D, H2], dt.bfloat16)
    b1_d = inp("b1", [1, H2], dt.bfloat16)
    W2_d = inp("W2", [H2, OUT_DIM], dt.bfloat16)
    b2_d = inp("b2", [1, OUT_DIM], dt.bfloat16)

    out_d = nc.dram_tensor("out", [N_GRAPHS, OUT_DIM], dt.float32,
                           kind="ExternalOutput")

    # hs_shard rows are (partition, block)-major: the SBUF [p, (b f)] layout
    # maps to a fully contiguous DMA (16KB descriptors instead of 256B)
    hs_shard = [nc.dram_tensor(f"hs_shard{i}", [128, NBLK, D], dt.bfloat16)
                for i in range(2)]
    hs_full = [nc.dram_tensor(f"hs_full{i}", [NT, D], dt.bfloat16,
                              addr_space="Shared") for i in range(2)]
    out_shard = nc.dram_tensor("out_shard", [GPC, OUT_DIM], dt.float32)
    out_full = nc.dram_tensor("out_full", [N_GRAPHS, OUT_DIM], dt.float32,
                              addr_space="Shared")
    rg = [list(range(NCORES))]

    def allgather(nc, src_dram, dst_dram):
        src = src_dram.ap()
        if len(src_dram.shape) == 3:
            src = src.rearrange("p b f -> (p b) f")
        if TLSIM:
            # timing stand-in: DMA the shard into its slice of the full table
            nc.sync.dma_start(out=dst_dram.ap()[0:src.shape[0], :], in_=src)
        else:
            nc.gpsimd.collective_compute(
                "AllGather", OP.bypass, replica_groups=rg,
                ins=[src], outs=[dst_dram.ap()])

    BLK_W = NBLK // WCHUNKS    # blocks per hs_shard write chunk

    with tile.TileContext(nc) as tc:
        with ExitStack() as ctx:
            const = ctx.enter_context(tc.tile_pool(name="const", bufs=1))
            ps_pool = ctx.enter_context(
                tc.tile_pool(name="ps", bufs=int(os.environ.get("KERNEL_PSBUFS", "3")),
                             space="PSUM"))
            pssum_pool = ctx.enter_context(
                tc.tile_pool(name="pssum", bufs=1, space="PSUM"))
            pstail_pool = ctx.enter_context(
                tc.tile_pool(name="pstail", bufs=int(os.environ.get("KERNEL_PTBUFS", "4")), space="PSUM"))
            oh_pool = ctx.enter_context(tc.tile_pool(name="oh", bufs=int(os.environ.get("KERNEL_OHBUFS", "24"))))
            tmp_pool = ctx.enter_context(tc.tile_pool(name="tmp", bufs=int(os.environ.get("KERNEL_TMPBUFS", "8"))))
            glo_pool = ctx.enter_context(tc.tile_pool(
                name="glo", bufs=int(os.environ.get("KERNEL_GBUFS", "3"))))
            small = ctx.enter_context(tc.tile_pool(name="small", bufs=1))

            # ------- resident constants
            h_sb = const.tile([128, NPC], dt.float32)
            hsall_sb = const.tile([128, NPC], dt.bfloat16)
            xT_sb = const.tile([128, NPC], dt.bfloat16)
            idxlo_sb = const.tile([128, NLO // 16], dt.int16)
            idxhi_sb = const.tile([128, NHI // 16], dt.int16)
            colloc_sb = const.tile([128, NTILES], dt.float32)
            scal_sb = const.tile([128, _C_F32], dt.float32)
            bfc_sb = const.tile([128, _B_BF], dt.bfloat16)

            dis_sb = scal_sb[:, _C_DIS:_C_DIS + NBLK]
            a_sb = scal_sb[:, _C_A:_C_A + NBLK]
            b_sb = scal_sb[:, _C_B:_C_B + NBLK]
            padneg_sb = scal_sb[:, _C_PAD:_C_PAD + NBLK]
            ident_sb = scal_sb[:, _C_ID:_C_ID + 128]
            embb_sb = scal_sb[:, _C_EMBB:_C_EMBB + 128]
            emask_sb = scal_sb[:, _C_EMASK:_C_EMASK + GPC]
            invcnt_sb = scal_sb[0:GPC, _C_INV:_C_INV + 1]
            ghot_sb = bfc_sb[:, _B_GHOT:_B_GHOT + NBLK * GPC]
            iota_sb = bfc_sb[:, _B_IOTA:_B_IOTA + 128]
            embW_sb = bfc_sb[:, _B_EMBW:_B_EMBW + 128]
            identbf_sb = bfc_sb[:, _B_IDBF:_B_IDBF + 128]

            # phase-1 deps (scal/bfc/xT) load first; gather tables after
            nc.sync.dma_start(out=scal_sb[:], in_=scal_d.ap())
            nc.sync.dma_start(out=bfc_sb[:], in_=bfc_d.ap())
            for c in range(4):
                csl = slice(c * (NPC // 4), (c + 1) * (NPC // 4))
                nc.sync.dma_start(out=xT_sb[:, csl], in_=xT_d.ap()[:, csl])
            for t, d in [(idxlo_sb, idxlo16_d), (idxhi_sb, idxhi16_d),
                         (colloc_sb, colloc_d)]:
                nc.sync.dma_start(out=t[:], in_=d.ap())

            # small-pool loads for the tail (issued early; SP queue is idle
            # during the iterations)
            ones_sb = small.tile([1, 128], dt.bfloat16)
            nc.sync.dma_start(out=ones_sb[:], in_=ones_d.ap())
            W1_sb = small.tile([128, 3, H2], dt.bfloat16)
            nc.sync.dma_start(out=W1_sb[:, :, :],
                              in_=W1_d.ap().rearrange("(c k) m -> k c m", k=128))
            b1_sb = small.tile([1, H2], dt.bfloat16)
            nc.sync.dma_start(out=b1_sb[:], in_=b1_d.ap())
            W2a_sb = small.tile([128, OUT_DIM], dt.bfloat16)
            nc.sync.dma_start(out=W2a_sb[:], in_=W2_d.ap()[0:128, :])
            W2b_sb = small.tile([H2 - 128, OUT_DIM], dt.bfloat16)
            nc.sync.dma_start(out=W2b_sb[:], in_=W2_d.ap()[128:H2, :])
            b2_sb = small.tile([1, OUT_DIM], dt.bfloat16)
            nc.sync.dma_start(out=b2_sb[:], in_=b2_d.ap())

            def write_hs_chunk(buf_idx, c):
                csl = slice(c * BLK_W * 128, (c + 1) * BLK_W * 128)
                nc.sync.dma_start(
                    out=hs_shard[buf_idx].ap()[:, c * BLK_W:(c + 1) * BLK_W, :],
                    in_=hsall_sb[:, csl].rearrange("p (b f) -> p b f", f=D))

            # ------- phase 1: h0 = x @ embW + embb ; hs0 = dis * h0
            for b in range(NBLK):
                bsl = slice(b * 128, (b + 1) * 128)
                ps = ps_pool.tile([128, D], dt.float32)
                nc.tensor.matmul(out=ps[:], lhsT=xT_sb[:, bsl], rhs=embW_sb,
                                 start=True, stop=True)
                nc.vector.tensor_tensor(out=h_sb[:, bsl], in0=ps[:],
                                        in1=embb_sb, op=OP.add)
                if b % 2 == 0:
                    nc.scalar.activation(out=hsall_sb[:, bsl], in_=h_sb[:, bsl],
                                         func=ACTF.Identity,
                                         scale=dis_sb[:, b:b + 1])
                else:
                    nc.vector.tensor_scalar(out=hsall_sb[:, bsl], in0=h_sb[:, bsl],
                                            scalar1=dis_sb[:, b:b + 1],
                                            scalar2=None, op0=OP.mult)
                if (b + 1) % BLK_W == 0:
                    write_hs_chunk(0, b // BLK_W)
            allgather(nc, hs_shard[0], hs_full[0])

            # tiles reserved for the fused pooling tail
            gmax = [const.tile([128, 128], dt.bfloat16, name=f"gmax{gg}")
                    for gg in range(GPC)]
            ps_sum = pssum_pool.tile([GPC, D], dt.float32, tag="pssum")
            pm = small.tile([128, GPC], dt.float32)

            # ------- phase 2: propagation iterations
            def chunks_of(n_tiles):
                out = []
                t = 0
                while n_tiles - t > TPC:
                    out.append((t, TPC))
                    t += TPC
                if n_tiles > t:
                    out.append((t, n_tiles - t))
                return out
            ch_lo = chunks_of(prep.nt_lo)
            ch_hi = chunks_of(prep.nt_hi)
            # tile position -> (chunk index, offset within chunk)
            def tile_map(chunks):
                m = {}
                for ci, (t0, n) in enumerate(chunks):
                    for i in range(n):
                        m[t0 + i] = (ci, i)
                return m
            tm_lo = tile_map(ch_lo)
            tm_hi = tile_map(ch_hi)

            for it in range(ITERATIONS):
                last = it == ITERATIONS - 1
                tbl = hs_full[it % 2]
                lo_tiles, hi_tiles = [], []
                # dma_gather: CHUNK idxs per instruction (short last chunk),
                # lo/hi interleaved
                for c in range(max(len(ch_lo), len(ch_hi))):
                    if c < len(ch_lo):
                        t0c, n = ch_lo[c]
                        gt = glo_pool.tile([128, TPC, D], dt.bfloat16,
                                           tag="glo")
                        nc.gpsimd.dma_gather(
                            out_ap=gt[:, 0:n, :], in_ap=tbl.ap()[0:HALF, :],
                            idxs_ap=idxlo_sb[:, t0c * 8:t0c * 8 + n * 8],
                            num_idxs=n * 128, num_idxs_reg=n * 128,
                            elem_size=D, single_packet=False)
                        lo_tiles.append(gt)
                    if c < len(ch_hi):
                        t0c, n = ch_hi[c]
                        gt = glo_pool.tile([128, TPC, D], dt.bfloat16,
                                           tag="ghi")
                        nc.gpsimd.dma_gather(
                            out_ap=gt[:, 0:n, :], in_ap=tbl.ap()[HALF:NT, :],
                            idxs_ap=idxhi_sb[:, t0c * 8:t0c * 8 + n * 8],
                            num_idxs=n * 128, num_idxs_reg=n * 128,
                            elem_size=D, single_packet=False)
                        hi_tiles.append(gt)

                for b in range(NBLK):
                    bsl = slice(b * 128, (b + 1) * 128)
                    tiles = prep.block_tiles[b]
                    if tiles:
                        ps = ps_pool.tile([128, D], dt.float32)
                        for j, (s, spos, gidx) in enumerate(tiles):
                            oh = oh_pool.tile([128, 128], dt.bfloat16)
                            eng = (nc.gpsimd if (OHSHARE > 0
                                                 and j % OHSHARE == OHSHARE - 1)
                                   else nc.vector)
                            eng.tensor_scalar(
                                out=oh[:], in0=iota_sb,
                                scalar1=colloc_sb[:, gidx:gidx + 1],
                                scalar2=None, op0=OP.is_equal)
                            tl = lo_tiles if s == 0 else hi_tiles
                            c, slot = (tm_lo if s == 0 else tm_hi)[spos]
                            rhs = tl[c][:, slot, :]
                            nc.tensor.matmul(
                                out=ps[:], lhsT=oh[:], rhs=rhs,
                                start=(j == 0), stop=(j == len(tiles) - 1))
                        u = tmp_pool.tile([128, 128], dt.float32)
                        nc.scalar.activation(
                            out=u[:], in_=h_sb[:, bsl], func=ACTF.Identity,
                            scale=a_sb[:, b:b + 1])
                        nc.vector.scalar_tensor_tensor(
                            out=h_sb[:, bsl], in0=ps[:], scalar=b_sb[:, b:b + 1],
                            in1=u[:], op0=OP.mult, op1=OP.add)
                    else:
                        nc.vector.tensor_scalar(
                            out=h_sb[:, bsl], in0=h_sb[:, bsl],
                            scalar1=a_sb[:, b:b + 1], scalar2=None, op0=OP.mult)
                    if not last:
                        nc.scalar.activation(
                            out=hsall_sb[:, bsl], in_=h_sb[:, bsl],
                            func=ACTF.Identity, scale=dis_sb[:, b:b + 1])
                        if (b + 1) % BLK_W == 0:
                            write_hs_chunk((it + 1) % 2, b // BLK_W)
                    else:
                        # fused tail: tanh + tri-pooling contributions.
                        # bias=-30 at pad slots keeps them out of the max
                        # (pre- or post-tanh, both < any real tanh value);
                        # ghot already masks pads out of the sum.
                        t0 = tmp_pool.tile([128, 128], dt.bfloat16, tag="t0")
                        nc.scalar.activation(out=t0[:], in_=h_sb[:, bsl],
                                             func=ACTF.Tanh,
                                             bias=padneg_sb[:, b:b + 1])
                        nc.tensor.matmul(out=ps_sum[:],
                                         lhsT=ghot_sb[:, b * GPC:(b + 1) * GPC],
                                         rhs=t0[:],
                                         start=(b == 0), stop=(b == NBLK - 1))
                        gg, bb = divmod(b, NBLK // GPC)
                        if bb == 0:
                            nc.vector.tensor_copy(out=gmax[gg][:], in_=t0[:])
                        else:
                            nc.vector.tensor_tensor(out=gmax[gg][:],
                                                    in0=gmax[gg][:],
                                                    in1=t0[:], op=OP.max)
                        if bb == NBLK // GPC - 1:
                            # graph gg's blocks all folded: finish its max
                            # (transpose to feature-partitions + reduce) now,
                            # overlapped with the remaining blocks
                            pst = pstail_pool.tile([128, 128], dt.bfloat16,
                                                   tag="tail")
                            nc.tensor.transpose(out=pst[:], in_=gmax[gg][:],
                                                identity=identbf_sb)
                            nc.vector.tensor_reduce(
                                out=pm[:, gg:gg + 1], in_=pst[:], axis=AX.X,
                                op=OP.max)
                if not last:
                    allgather(nc, hs_shard[(it + 1) % 2], hs_full[(it + 1) % 2])

            # ------- phase 3/4: finish pooling + MLP on the LOCAL 8 graphs
            # (pool stats never leave the core; only the final [GPC, 32]
            # outputs are all-gathered)
            nc.vector.tensor_tensor(out=pm[:], in0=pm[:], in1=emask_sb,
                                    op=OP.mult)
            pmT = pstail_pool.tile([GPC, 128], dt.float32, tag="tail")
            nc.tensor.transpose(out=pmT[:], in_=pm[:], identity=ident_sb)

            gf = small.tile([GPC, 3 * D], dt.bfloat16)
            nc.vector.tensor_copy(out=gf[:, 0:D], in_=ps_sum[:])
            nc.vector.tensor_copy(out=gf[:, D:2 * D], in_=pmT[:])
            nc.vector.tensor_scalar(out=gf[:, 2 * D:3 * D], in0=ps_sum[:],
                                    scalar1=invcnt_sb, scalar2=None,
                                    op0=OP.mult)

            gfT = []
            for c in range(3):
                pt = pstail_pool.tile([128, GPC], dt.bfloat16, tag="tail")
                nc.tensor.transpose(out=pt[:], in_=gf[:, c * D:(c + 1) * D],
                                    identity=identbf_sb[0:GPC, 0:GPC])
                st = small.tile([128, GPC], dt.bfloat16)
                nc.vector.tensor_copy(out=st[:], in_=pt[:])
                gfT.append(st)

            ps1 = pstail_pool.tile([GPC, H2], dt.float32, tag="tail")
            for c in range(3):
                nc.tensor.matmul(out=ps1[:], lhsT=gfT[c][:],
                                 rhs=W1_sb[:, c, :], start=(c == 0), stop=False)
            nc.tensor.matmul(out=ps1[:], lhsT=ones_sb[:, 0:GPC],
                             rhs=b1_sb[:], start=False, stop=True)
            g1 = small.tile([GPC, H2], dt.bfloat16)
            nc.scalar.activation(out=g1[:], in_=ps1[:], func=ACTF.Lrelu)

            g1T = []
            for c, w in [(0, 128), (1, H2 - 128)]:
                pt = pstail_pool.tile([128, GPC], dt.bfloat16, tag="tail")
                nc.tensor.transpose(out=pt[0:w, :], in_=g1[:, c * 128:c * 128 + w],
                                    identity=identbf_sb[0:GPC, 0:GPC])
                st = small.tile([128, GPC], dt.bfloat16)
                nc.vector.tensor_copy(out=st[0:w, :], in_=pt[0:w, :])
                g1T.append(st)

            ps2 = pstail_pool.tile([GPC, OUT_DIM], dt.float32, tag="tail")
            nc.tensor.matmul(out=ps2[:], lhsT=g1T[0][:],
                             rhs=W2a_sb[:], start=True, stop=False)
            nc.tensor.matmul(out=ps2[:], lhsT=g1T[1][0:H2 - 128, :],
                             rhs=W2b_sb[:], start=False, stop=False)
            nc.tensor.matmul(out=ps2[:], lhsT=ones_sb[:, 0:GPC],
                             rhs=b2_sb[:], start=False, stop=True)
            o_sb = small.tile([GPC, OUT_DIM], dt.float32)
            nc.scalar.activation(out=o_sb[:], in_=ps2[:], func=ACTF.Lrelu)
            nc.sync.dma_start(out=out_shard.ap(), in_=o_sb[:])
            allgather(nc, out_shard, out_full)
            nc.sync.dma_start(out=out_d.ap(), in_=out_full.ap())

    nc.compile()
    return nc


# ---------------------------------------------------------------- entry
_CACHE = {}


def kernel(x, edge_index, batch, emb_W, emb_b, W1, b1, W2, b2):
    prep = preprocess(x, edge_index, batch, emb_W, emb_b, W1, b1, W2, b2)
    key = (prep.n_lo, prep.n_hi, prep.ntiles,
           tuple(len(bt) for bt in prep.block_tiles))
    nc = _CACHE.get(key)
    if nc is None:
        nc = build_program(prep)
        _CACHE[key] = nc
    res = bass_utils.run_bass_kernel_spmd(
        nc, prep.in_maps, core_ids=list(range(NCORES)),
        trace=False)
    kernel.last_results = res
    out_pos = np.asarray(res.results[0]["out"], np.float32)
    return np.ascontiguousarray(out_pos[prep.pos_of])
